# revision 16
# baseline (speedup 1.0000x reference)
"""Trainium2 Bass kernel for nn_CIN (xDeepFM compressed-interaction network).

Math: each CIN layer computes, per sample b and feature-dim d (a "column"
n=(b,d)):  y[o] = sum_{h,m} W[o,h,m] * a[h] * b[m]  — a bilinear form.

We avoid materializing the outer-product tensor z[h*m, n] (which needs slow
cross-partition broadcasts) by polarization:  a*b = ((a+b)^2 - a^2 - b^2)/2.
Each layer becomes:  s = V @ t   (pair sums, TensorE)
                     q = s*s     (elementwise square)
                     y = C @ q + G @ t^2   (TensorE, PSUM-accumulated)
with V a 0/1 pair-selection matrix and C,G folded from W host-side (exact).

Layer 0 uses the symmetric fold (741 unordered pairs of 39 features);
layer 1 uses all 64*39=2496 (nh,x) pairs.  Everything on-device is fp16
(inputs/weights) with fp32 PSUM accumulation.

Engine balance (per 512-column tile; 13 wide square-pairs total):
  PE   : 54 matmuls (V0 6, C0 6, G0 1, V1 20, C1 20, G1 1)  ~= 11.5us
  ACT  : 3 relu(+bias) + 8 wide squares (PSUM->SBUF)         ~= 10.1us
  DVE  : 2 sq pairs (copy+mul), 3 pair copies (for Pool),
         2 d-sum reduces                                     ~= 8.3us
  Pool : 3 pair muls + x^2 and nh^2 muls (all SBUF f16)      ~= 8.6us
(DVE TensorTensor cannot read PSUM twice — walrus verifier rejects it —
so non-ACT squares go copy-then-multiply; Pool has no PSUM port at all.)
V outputs are written pairwise into [128,1024] two-bank PSUM tiles so each
square instruction covers two chunks (amortizes the fixed access latency).
PSUM budget: 3x2 (V pairs) + 1 (y0) + 1 (y1) = 8 banks.

Sharding: pure data parallel — batch 4096 split as 512 per NeuronCore
across 8 cores; weights replicated.
"""

import numpy as np

B, F, D = 4096, 39, 16
L0, L1 = 128, 128
H1 = L0 // 2                      # 64 hidden maps feed layer 1
NCORES = 8
BL = B // NCORES                  # 512 samples per core
NCOL = BL * D                     # 8192 columns per core
NT = 512                          # columns per tile
NTILES = NCOL // NT               # 16
NB = NT // D                      # samples per tile (32)

K0 = F * (F - 1) // 2             # 741 layer-0 pairs
K1 = H1 * F                       # 2496 layer-1 pairs
NC0 = (K0 + 127) // 128           # 6 chunks (K0 padded to 768)
NC1 = (K1 + 127) // 128           # 20 chunks (K1 padded to 2560)
NP0 = NC0 // 2                    # 3 wide chunk-pairs
NP1 = NC1 // 2                    # 10 wide chunk-pairs
T1 = 128                          # t rows: [x 0:39 | zeros 39:64 | nh 64:128]
NH0 = 64                          # nh base partition in t


def _host_weights(W0, b0, W1, b1):
    """Fold W0/W1 into the square-trick operands (all exact, fp32)."""
    W0 = np.asarray(W0, np.float32)
    W1 = np.asarray(W1, np.float32)
    S0 = W0.reshape(L0, F, F)
    S0 = (S0 + S0.transpose(0, 2, 1)) / 2
    iu = np.triu_indices(F, 1)                       # 741 (h<m) pairs
    V0 = np.zeros((128 * NC0, F), np.float32)
    V0[np.arange(K0), iu[0]] = 1
    V0[np.arange(K0), iu[1]] = 1
    C0 = np.zeros((L0, 128 * NC0), np.float32)
    C0[:, :K0] = S0[:, iu[0], iu[1]]
    rowsum = S0.sum(2)
    G0 = np.einsum('ohh->oh', S0) * 2 - rowsum       # S[h,h] - sum_{m!=h} S[h,m]

    B1 = W1.reshape(L1, H1, F)
    hh, mm = np.meshgrid(np.arange(H1), np.arange(F), indexing='ij')
    hh, mm = hh.ravel(), mm.ravel()                  # 2496 pairs, h-major
    V1 = np.zeros((128 * NC1, T1), np.float32)
    V1[np.arange(K1), mm] = 1                        # x part at rows 0:39
    V1[np.arange(K1), NH0 + hh] = 1                  # nh part at rows 64:128
    C1 = np.zeros((L1, 128 * NC1), np.float32)
    C1[:, :K1] = B1[:, hh, mm] / 2
    G1 = np.zeros((L1, T1), np.float32)
    G1[:, :F] = -B1.sum(1) / 2                       # coeff on x^2
    G1[:, NH0:] = -B1.sum(2) / 2                     # coeff on nh^2

    def blockT(C):
        # [128, K] -> per-128-column-block transpose: lhsT[k, o] = C[o, base+k]
        L, K = C.shape
        return np.ascontiguousarray(
            C.reshape(L, K // 128, 128).transpose(2, 1, 0).reshape(128, -1)
        )

    return {
        "V0T": V0.T.astype(np.float16),              # [39, 768]
        "V1T": V1.T.astype(np.float16),              # [128, 2560]
        "C0T": blockT(C0).astype(np.float16),        # [128, 768]   (lhsT chunks)
        "C1T": blockT(C1).astype(np.float16),        # [128, 2560]
        "G0T": G0.T.astype(np.float16),              # [39, 128]
        "G1T": G1.T.astype(np.float16),              # [128, 128]
        "b0": np.asarray(b0, np.float32).reshape(L0, 1),
        "b1": np.asarray(b1, np.float32).reshape(L1, 1),
    }


_NC_CACHE = {}


def _build_nc(repeat=1):
    key = ("nc", repeat)
    if key in _NC_CACHE:
        return _NC_CACHE[key]
    from contextlib import ExitStack
    import concourse.bacc as bacc
    import concourse.mybir as mybir
    import concourse.tile as tile

    f16 = mybir.dt.float16
    f32 = mybir.dt.float32

    nc = bacc.Bacc("TRN2", target_bir_lowering=False, debug=False)

    xT_d = nc.dram_tensor("xT", [F, NCOL], f16, kind="ExternalInput")
    V0T_d = nc.dram_tensor("V0T", [F, 128 * NC0], f16, kind="ExternalInput")
    V1T_d = nc.dram_tensor("V1T", [T1, 128 * NC1], f16, kind="ExternalInput")
    C0T_d = nc.dram_tensor("C0T", [128, 128 * NC0], f16, kind="ExternalInput")
    C1T_d = nc.dram_tensor("C1T", [128, 128 * NC1], f16, kind="ExternalInput")
    G0T_d = nc.dram_tensor("G0T", [F, 128], f16, kind="ExternalInput")
    G1T_d = nc.dram_tensor("G1T", [T1, 128], f16, kind="ExternalInput")
    b0_d = nc.dram_tensor("b0", [L0, 1], f32, kind="ExternalInput")
    b1_d = nc.dram_tensor("b1", [L1, 1], f32, kind="ExternalInput")
    out_d = nc.dram_tensor("out", [L0 - H1 + L1, BL], f32, kind="ExternalOutput")

    Relu = mybir.ActivationFunctionType.Relu
    Square = mybir.ActivationFunctionType.Square

    with tile.TileContext(nc) as tc, ExitStack() as ctx:
        const = ctx.enter_context(tc.tile_pool(name="const", bufs=1))
        tp = ctx.enter_context(tc.tile_pool(name="tp", bufs=1))
        sqp = ctx.enter_context(tc.tile_pool(name="sqp", bufs=4))
        scrp = ctx.enter_context(tc.tile_pool(name="scrp", bufs=2))
        rp = ctx.enter_context(tc.tile_pool(name="rp", bufs=2))
        outp = ctx.enter_context(tc.tile_pool(name="outp", bufs=1))
        sps = ctx.enter_context(tc.tile_pool(name="sps", bufs=3, space="PSUM"))
        yps0 = ctx.enter_context(tc.tile_pool(name="yps0", bufs=1, space="PSUM"))
        yps1 = ctx.enter_context(tc.tile_pool(name="yps1", bufs=1, space="PSUM"))

        # resident weights
        V0T = const.tile([F, 128 * NC0], f16)
        V1T = const.tile([T1, 128 * NC1], f16)
        C0T = const.tile([128, 128 * NC0], f16)
        C1T = const.tile([128, 128 * NC1], f16)
        G0T = const.tile([F, 128], f16)
        G1T = const.tile([T1, 128], f16)
        b0t = const.tile([L0, 1], f32)
        b1t = const.tile([L1, 1], f32)
        for dst, src in ((V0T, V0T_d), (V1T, V1T_d), (C0T, C0T_d),
                         (C1T, C1T_d), (G0T, G0T_d), (G1T, G1T_d),
                         (b0t, b0_d), (b1t, b1_d)):
            nc.sync.dma_start(out=dst[:], in_=src.ap())

        # persistent double-buffered t = [x; 0; nh] and t2 = [x^2; 0; nh^2]
        t_bufs = [tp.tile([T1, NT], f16, name=f"t{i}", tag=f"t{i}")
                  for i in range(2)]
        t2_bufs = [tp.tile([T1, NT], f16, name=f"t2_{i}", tag=f"t2_{i}")
                   for i in range(2)]
        for tt in (*t_bufs, *t2_bufs):
            nc.vector.memset(tt[32:NH0, :], 0.0)     # one-time zero padding

        out0 = outp.tile([H1, BL], f32)
        out1 = outp.tile([L1, BL], f32)

        # Wide square of a [128, 2*NT] PSUM pair, dispatched per engine plan:
        #   'A': ACT square straight from PSUM
        #   'D': DVE copy to SBUF f16, DVE multiply
        #   'P': DVE copy to SBUF f16, Pool multiply
        def square(dst, src, how, scratch):
            if how == 'A':
                nc.scalar.square(dst, src)
            else:
                nc.vector.tensor_copy(scratch[:], src)
                eng = nc.vector if how == 'D' else nc.gpsimd
                eng.tensor_mul(dst, scratch[:], scratch[:])

        for it, nt in enumerate(
                [nt for _ in range(repeat) for nt in range(NTILES)]):
            csl = slice(nt * NT, (nt + 1) * NT)
            t = t_bufs[it % 2]
            t2 = t2_bufs[it % 2]
            nc.sync.dma_start(out=t[0:F, :], in_=xT_d.ap()[:, csl])
            nc.vector.tensor_mul(t2[0:F, :], t[0:F, :], t[0:F, :])   # x^2

            # ---- layer 0: s0 = V0 @ x ; q0 = s0^2 (3 wide pairs) ----
            PLAN0 = "ADA"
            sq0 = []
            for p in range(NP0):
                ps = sps.tile([128, 2 * NT], f32)
                for h in range(2):
                    i = 2 * p + h
                    nc.tensor.matmul(ps[:, h * NT:(h + 1) * NT],
                                     V0T[:, i * 128:(i + 1) * 128],
                                     t[0:F, :], start=True, stop=True)
                sq = sqp.tile([128, 2 * NT], f16)
                scr = (scrp.tile([128, 2 * NT], f16, name="scr0")
                       if PLAN0[p] != "A" else None)
                square(sq[:], ps[:], PLAN0[p], scr)
                sq0.append(sq)

            # ---- y0 = G0 @ x^2 + C0 @ q0  (G0 first: x^2 is ready early) ----
            y0 = yps0.tile([L0, NT], f32)
            nc.tensor.matmul(y0[:], G0T[:], t2[0:F, :], start=True, stop=False)
            for i in range(NC0):
                nc.tensor.matmul(y0[:], C0T[:, i * 128:(i + 1) * 128],
                                 sq0[i // 2][:, (i % 2) * NT:(i % 2 + 1) * NT],
                                 start=False, stop=(i == NC0 - 1))

            # relu + split: nh feeds layer 1, r0 is direct-connect
            nc.scalar.activation(t[NH0:T1, :], y0[0:H1, :], Relu, bias=b0t[0:H1])
            r0 = rp.tile([H1, NT], f32, tag="r0")
            nc.scalar.activation(r0[:], y0[H1:L0, :], Relu, bias=b0t[H1:L0])
            nc.vector.tensor_mul(t2[NH0:T1, :], t[NH0:T1, :], t[NH0:T1, :])  # nh^2

            # ---- layer 1: s1 = V1 @ [x; nh] ; q1 = s1^2 (10 wide pairs) ----
            # Pool handles the first three pairs (produced earliest, and their
            # C1 matmuls are deferred to the end of the accumulation, so the
            # slow Pool multiplies never stall the in-order PE queue).
            PLAN1 = "PPPADAAAAA"
            sq1 = []
            for p in range(NP1):
                ps = sps.tile([128, 2 * NT], f32)
                for h in range(2):
                    i = 2 * p + h
                    nc.tensor.matmul(ps[:, h * NT:(h + 1) * NT],
                                     V1T[:, i * 128:(i + 1) * 128],
                                     t[:], start=True, stop=True)
                sq = sqp.tile([128, 2 * NT], f16)
                scr = (scrp.tile([128, 2 * NT], f16, name="scr1")
                       if PLAN1[p] != "A" else None)
                square(sq[:], ps[:], PLAN1[p], scr)
                sq1.append(sq)

            # ---- y1 = C1 @ q1 + G1 @ t^2 (Pool-squared chunks last) ----
            y1 = yps1.tile([L1, NT], f32)
            order = [i for i in range(NC1) if PLAN1[i // 2] != 'P']
            order += [i for i in range(NC1) if PLAN1[i // 2] == 'P']
            for n, i in enumerate(order):
                nc.tensor.matmul(y1[:], C1T[:, i * 128:(i + 1) * 128],
                                 sq1[i // 2][:, (i % 2) * NT:(i % 2 + 1) * NT],
                                 start=(n == 0), stop=(n == NC1 - 1))
                if n == 0:
                    nc.tensor.matmul(y1[:], G1T[:], t2[:],
                                     start=False, stop=False)

            r1 = rp.tile([L1, NT], f32, tag="r1")
            nc.scalar.activation(r1[:], y1[:], Relu, bias=b1t[:])

            # ---- sum over d (innermost 16 of each column group) ----
            bsl = slice(nt * NB, (nt + 1) * NB)
            nc.vector.tensor_reduce(
                out0[:, bsl], r0[:].rearrange("p (b d) -> p b d", d=D),
                axis=mybir.AxisListType.X, op=mybir.AluOpType.add)
            nc.vector.tensor_reduce(
                out1[:, bsl], r1[:].rearrange("p (b d) -> p b d", d=D),
                axis=mybir.AxisListType.X, op=mybir.AluOpType.add)

        nc.sync.dma_start(out=out_d.ap()[0:H1, :], in_=out0[:])
        nc.sync.dma_start(out=out_d.ap()[H1:, :], in_=out1[:])

    nc.compile()
    _NC_CACHE[key] = nc
    return nc


def _run(inputs, trace=False):
    from concourse.bass_utils import run_bass_kernel_spmd

    x = np.asarray(inputs["x"], np.float32)
    w = _host_weights(inputs["W0"], inputs["b0"], inputs["W1"], inputs["b1"])
    nc = _build_nc()

    in_maps = []
    for c in range(NCORES):
        xs = x[c * BL:(c + 1) * BL]                          # [512, 39, 16]
        xT = np.ascontiguousarray(
            xs.transpose(1, 0, 2).reshape(F, NCOL)).astype(np.float16)
        m = {"xT": xT}
        m.update(w)
        in_maps.append(m)

    res = run_bass_kernel_spmd(nc, in_maps, core_ids=list(range(NCORES)),
                               trace=trace)
    out = np.empty((B, L0 - H1 + L1), np.float32)
    for c in range(NCORES):
        out[c * BL:(c + 1) * BL] = res.results[c]["out"].T
    return out, res


def kernel(**inputs):
    out, _ = _run(inputs)
    return out


# revision 17
# speedup vs baseline: 1.6951x; 1.6951x over previous
"""Trainium2 Bass kernel for nn_CIN (xDeepFM compressed-interaction network).

Math: each CIN layer computes, per sample b and feature-dim d (a "column"
n=(b,d)):  y[o] = sum_{h,m} W[o,h,m] * a[h] * b[m]  — a bilinear form.

We avoid materializing the outer-product tensor z[h*m, n] (which needs slow
cross-partition broadcasts) by polarization:  a*b = ((a+b)^2 - a^2 - b^2)/2.
Each layer becomes:  s = V @ t   (pair sums, TensorE)
                     q = s*s     (elementwise square)
                     y = C @ q + G @ t^2   (TensorE, PSUM-accumulated)
with V a 0/1 pair-selection matrix and C,G folded from W host-side (exact).

Layer 0 uses the symmetric fold (741 unordered pairs of 39 features);
layer 1 uses all 64*39=2496 (nh,x) pairs.  Everything on-device is fp16
(inputs/weights) with fp32 PSUM accumulation.

Engine balance (per 512-column tile; 13 wide square-pairs total):
  PE   : 54 matmuls (V0 6, C0 6, G0 1, V1 20, C1 20, G1 1)  ~= 11.5us
  ACT  : 3 relu(+bias) + 8 wide squares (PSUM->SBUF)         ~= 10.1us
  DVE  : 2 sq pairs (copy+mul), 3 pair copies (for Pool),
         2 d-sum reduces                                     ~= 8.3us
  Pool : 3 pair muls + x^2 and nh^2 muls (all SBUF f16)      ~= 8.6us
(DVE TensorTensor cannot read PSUM twice — walrus verifier rejects it —
so non-ACT squares go copy-then-multiply; Pool has no PSUM port at all.)
V outputs are written pairwise into [128,1024] two-bank PSUM tiles so each
square instruction covers two chunks (amortizes the fixed access latency).
PSUM budget: 3x2 (V pairs) + 1 (y0) + 1 (y1) = 8 banks.

Sharding: pure data parallel — batch 4096 split as 512 per NeuronCore
across 8 cores; weights replicated.
"""

import numpy as np

B, F, D = 4096, 39, 16
L0, L1 = 128, 128
H1 = L0 // 2                      # 64 hidden maps feed layer 1
NCORES = 8
BL = B // NCORES                  # 512 samples per core
NCOL = BL * D                     # 8192 columns per core
NT = 512                          # columns per tile
NTILES = NCOL // NT               # 16
NB = NT // D                      # samples per tile (32)

K0 = F * (F - 1) // 2             # 741 layer-0 pairs
K1 = H1 * F                       # 2496 layer-1 pairs
NC0 = (K0 + 127) // 128           # 6 chunks (K0 padded to 768)
NC1 = (K1 + 127) // 128           # 20 chunks (K1 padded to 2560)
NP0 = NC0 // 2                    # 3 wide chunk-pairs
NP1 = NC1 // 2                    # 10 wide chunk-pairs
T1 = 128                          # t rows: [x 0:39 | zeros 39:64 | nh 64:128]
NH0 = 64                          # nh base partition in t


def _host_weights(W0, b0, W1, b1):
    """Fold W0/W1 into the square-trick operands (all exact, fp32)."""
    W0 = np.asarray(W0, np.float32)
    W1 = np.asarray(W1, np.float32)
    S0 = W0.reshape(L0, F, F)
    S0 = (S0 + S0.transpose(0, 2, 1)) / 2
    iu = np.triu_indices(F, 1)                       # 741 (h<m) pairs
    V0 = np.zeros((128 * NC0, F), np.float32)
    V0[np.arange(K0), iu[0]] = 1
    V0[np.arange(K0), iu[1]] = 1
    C0 = np.zeros((L0, 128 * NC0), np.float32)
    C0[:, :K0] = S0[:, iu[0], iu[1]]
    rowsum = S0.sum(2)
    G0 = np.einsum('ohh->oh', S0) * 2 - rowsum       # S[h,h] - sum_{m!=h} S[h,m]

    B1 = W1.reshape(L1, H1, F)
    hh, mm = np.meshgrid(np.arange(H1), np.arange(F), indexing='ij')
    hh, mm = hh.ravel(), mm.ravel()                  # 2496 pairs, h-major
    V1 = np.zeros((128 * NC1, T1), np.float32)
    V1[np.arange(K1), mm] = 1                        # x part at rows 0:39
    V1[np.arange(K1), NH0 + hh] = 1                  # nh part at rows 64:128
    C1 = np.zeros((L1, 128 * NC1), np.float32)
    C1[:, :K1] = B1[:, hh, mm] / 2
    G1 = np.zeros((L1, T1), np.float32)
    G1[:, :F] = -B1.sum(1) / 2                       # coeff on x^2
    G1[:, NH0:] = -B1.sum(2) / 2                     # coeff on nh^2

    def blockT(C):
        # [128, K] -> per-128-column-block transpose: lhsT[k, o] = C[o, base+k]
        L, K = C.shape
        return np.ascontiguousarray(
            C.reshape(L, K // 128, 128).transpose(2, 1, 0).reshape(128, -1)
        )

    return {
        "V0T": V0.T.astype(np.float16),              # [39, 768]
        "V1T": V1.T.astype(np.float16),              # [128, 2560]
        "C0T": blockT(C0).astype(np.float16),        # [128, 768]   (lhsT chunks)
        "C1T": blockT(C1).astype(np.float16),        # [128, 2560]
        "G0T": G0.T.astype(np.float16),              # [39, 128]
        "G1T": G1.T.astype(np.float16),              # [128, 128]
        "b0": np.asarray(b0, np.float32).reshape(L0, 1),
        "b1": np.asarray(b1, np.float32).reshape(L1, 1),
    }


_NC_CACHE = {}


def _build_nc(repeat=1):
    key = ("nc", repeat)
    if key in _NC_CACHE:
        return _NC_CACHE[key]
    from contextlib import ExitStack
    import concourse.bacc as bacc
    import concourse.mybir as mybir
    import concourse.tile as tile

    f16 = mybir.dt.float16
    f32 = mybir.dt.float32

    nc = bacc.Bacc("TRN2", target_bir_lowering=False, debug=False)

    xT_d = nc.dram_tensor("xT", [F, NCOL], f16, kind="ExternalInput")
    V0T_d = nc.dram_tensor("V0T", [F, 128 * NC0], f16, kind="ExternalInput")
    V1T_d = nc.dram_tensor("V1T", [T1, 128 * NC1], f16, kind="ExternalInput")
    C0T_d = nc.dram_tensor("C0T", [128, 128 * NC0], f16, kind="ExternalInput")
    C1T_d = nc.dram_tensor("C1T", [128, 128 * NC1], f16, kind="ExternalInput")
    G0T_d = nc.dram_tensor("G0T", [F, 128], f16, kind="ExternalInput")
    G1T_d = nc.dram_tensor("G1T", [T1, 128], f16, kind="ExternalInput")
    b0_d = nc.dram_tensor("b0", [L0, 1], f32, kind="ExternalInput")
    b1_d = nc.dram_tensor("b1", [L1, 1], f32, kind="ExternalInput")
    out_d = nc.dram_tensor("out", [L0 - H1 + L1, BL], f32, kind="ExternalOutput")

    Relu = mybir.ActivationFunctionType.Relu
    Square = mybir.ActivationFunctionType.Square

    with tile.TileContext(nc) as tc, ExitStack() as ctx:
        const = ctx.enter_context(tc.tile_pool(name="const", bufs=1))
        tp = ctx.enter_context(tc.tile_pool(name="tp", bufs=1))
        sqp = ctx.enter_context(tc.tile_pool(name="sqp", bufs=6))
        scrp = ctx.enter_context(tc.tile_pool(name="scrp", bufs=4))
        rp = ctx.enter_context(tc.tile_pool(name="rp", bufs=2))
        outp = ctx.enter_context(tc.tile_pool(name="outp", bufs=1))
        sps = ctx.enter_context(tc.tile_pool(name="sps", bufs=3, space="PSUM"))
        yps0 = ctx.enter_context(tc.tile_pool(name="yps0", bufs=1, space="PSUM"))
        yps1 = ctx.enter_context(tc.tile_pool(name="yps1", bufs=1, space="PSUM"))

        # resident weights
        V0T = const.tile([F, 128 * NC0], f16)
        V1T = const.tile([T1, 128 * NC1], f16)
        C0T = const.tile([128, 128 * NC0], f16)
        C1T = const.tile([128, 128 * NC1], f16)
        G0T = const.tile([F, 128], f16)
        G1T = const.tile([T1, 128], f16)
        b0t = const.tile([L0, 1], f32)
        b1t = const.tile([L1, 1], f32)
        for dst, src in ((V0T, V0T_d), (V1T, V1T_d), (C0T, C0T_d),
                         (C1T, C1T_d), (G0T, G0T_d), (G1T, G1T_d),
                         (b0t, b0_d), (b1t, b1_d)):
            nc.sync.dma_start(out=dst[:], in_=src.ap())

        # persistent double-buffered t = [x; 0; nh] and t2 = [x^2; 0; nh^2]
        t_bufs = [tp.tile([T1, NT], f16, name=f"t{i}", tag=f"t{i}")
                  for i in range(2)]
        t2_bufs = [tp.tile([T1, NT], f16, name=f"t2_{i}", tag=f"t2_{i}")
                   for i in range(2)]
        for tt in (*t_bufs, *t2_bufs):
            nc.vector.memset(tt[32:NH0, :], 0.0)     # one-time zero padding

        out0 = outp.tile([H1, BL], f32)
        out1 = outp.tile([L1, BL], f32)

        # Wide square of a [128, 2*NT] PSUM pair, dispatched per engine plan:
        #   'A': ACT square straight from PSUM
        #   'D': DVE copy to SBUF f16, DVE multiply
        #   'P': DVE copy to SBUF f16, Pool multiply
        def square(dst, src, how, scratch):
            if how == 'A':
                nc.scalar.square(dst, src)
            else:
                nc.vector.tensor_copy(scratch[:], src)
                eng = nc.vector if how == 'D' else nc.gpsimd
                eng.tensor_mul(dst, scratch[:], scratch[:])

        for it, nt in enumerate(
                [nt for _ in range(repeat) for nt in range(NTILES)]):
            csl = slice(nt * NT, (nt + 1) * NT)
            t = t_bufs[it % 2]
            t2 = t2_bufs[it % 2]
            nc.sync.dma_start(out=t[0:F, :], in_=xT_d.ap()[:, csl])
            nc.vector.tensor_mul(t2[0:F, :], t[0:F, :], t[0:F, :])   # x^2

            # ---- layer 0: s0 = V0 @ x ; q0 = s0^2 (3 wide pairs) ----
            PLAN0 = "ADA"
            sq0 = []
            for p in range(NP0):
                ps = sps.tile([128, 2 * NT], f32)
                for h in range(2):
                    i = 2 * p + h
                    nc.tensor.matmul(ps[:, h * NT:(h + 1) * NT],
                                     V0T[:, i * 128:(i + 1) * 128],
                                     t[0:F, :], start=True, stop=True)
                sq = sqp.tile([128, 2 * NT], f16)
                scr = (scrp.tile([128, 2 * NT], f16, name="scr0")
                       if PLAN0[p] != "A" else None)
                square(sq[:], ps[:], PLAN0[p], scr)
                sq0.append(sq)

            # ---- y0 = G0 @ x^2 + C0 @ q0  (G0 first: x^2 is ready early) ----
            y0 = yps0.tile([L0, NT], f32)
            nc.tensor.matmul(y0[:], G0T[:], t2[0:F, :], start=True, stop=False)
            for i in range(NC0):
                nc.tensor.matmul(y0[:], C0T[:, i * 128:(i + 1) * 128],
                                 sq0[i // 2][:, (i % 2) * NT:(i % 2 + 1) * NT],
                                 start=False, stop=(i == NC0 - 1))

            # relu + split: nh feeds layer 1, r0 is direct-connect
            nc.scalar.activation(t[NH0:T1, :], y0[0:H1, :], Relu, bias=b0t[0:H1])
            r0 = rp.tile([H1, NT], f32, tag="r0")
            nc.scalar.activation(r0[:], y0[H1:L0, :], Relu, bias=b0t[H1:L0])
            nc.vector.tensor_mul(t2[NH0:T1, :], t[NH0:T1, :], t[NH0:T1, :])  # nh^2

            # ---- layer 1: s1 = V1 @ [x; nh] ; q1 = s1^2 (10 wide pairs) ----
            # Pool handles the first three pairs (produced earliest, and their
            # C1 matmuls are deferred to the end of the accumulation, so the
            # slow Pool multiplies never stall the in-order PE queue).
            PLAN1 = "PADAAAPADA"
            sq1 = []
            for p in range(NP1):
                ps = sps.tile([128, 2 * NT], f32)
                for h in range(2):
                    i = 2 * p + h
                    nc.tensor.matmul(ps[:, h * NT:(h + 1) * NT],
                                     V1T[:, i * 128:(i + 1) * 128],
                                     t[:], start=True, stop=True)
                sq = sqp.tile([128, 2 * NT], f16)
                scr = (scrp.tile([128, 2 * NT], f16, name="scr1")
                       if PLAN1[p] != "A" else None)
                square(sq[:], ps[:], PLAN1[p], scr)
                sq1.append(sq)

            # ---- y1 = C1 @ q1 + G1 @ t^2 (Pool-squared chunks last) ----
            y1 = yps1.tile([L1, NT], f32)
            order = [i for i in range(NC1) if PLAN1[i // 2] != 'P']
            order += [i for i in range(NC1) if PLAN1[i // 2] == 'P']
            for n, i in enumerate(order):
                nc.tensor.matmul(y1[:], C1T[:, i * 128:(i + 1) * 128],
                                 sq1[i // 2][:, (i % 2) * NT:(i % 2 + 1) * NT],
                                 start=(n == 0), stop=(n == NC1 - 1))
                if n == 0:
                    nc.tensor.matmul(y1[:], G1T[:], t2[:],
                                     start=False, stop=False)

            r1 = rp.tile([L1, NT], f32, tag="r1")
            nc.scalar.activation(r1[:], y1[:], Relu, bias=b1t[:])

            # ---- sum over d (innermost 16 of each column group) ----
            bsl = slice(nt * NB, (nt + 1) * NB)
            nc.vector.tensor_reduce(
                out0[:, bsl], r0[:].rearrange("p (b d) -> p b d", d=D),
                axis=mybir.AxisListType.X, op=mybir.AluOpType.add)
            nc.vector.tensor_reduce(
                out1[:, bsl], r1[:].rearrange("p (b d) -> p b d", d=D),
                axis=mybir.AxisListType.X, op=mybir.AluOpType.add)

        nc.sync.dma_start(out=out_d.ap()[0:H1, :], in_=out0[:])
        nc.sync.dma_start(out=out_d.ap()[H1:, :], in_=out1[:])

    nc.compile()
    _NC_CACHE[key] = nc
    return nc


def _run(inputs, trace=False):
    from concourse.bass_utils import run_bass_kernel_spmd

    x = np.asarray(inputs["x"], np.float32)
    w = _host_weights(inputs["W0"], inputs["b0"], inputs["W1"], inputs["b1"])
    nc = _build_nc()

    in_maps = []
    for c in range(NCORES):
        xs = x[c * BL:(c + 1) * BL]                          # [512, 39, 16]
        xT = np.ascontiguousarray(
            xs.transpose(1, 0, 2).reshape(F, NCOL)).astype(np.float16)
        m = {"xT": xT}
        m.update(w)
        in_maps.append(m)

    res = run_bass_kernel_spmd(nc, in_maps, core_ids=list(range(NCORES)),
                               trace=trace)
    out = np.empty((B, L0 - H1 + L1), np.float32)
    for c in range(NCORES):
        out[c * BL:(c + 1) * BL] = res.results[c]["out"].T
    return out, res


def kernel(**inputs):
    out, _ = _run(inputs)
    return out


# revision 19
# speedup vs baseline: 1.9234x; 1.1346x over previous
"""Trainium2 Bass kernel for nn_CIN (xDeepFM compressed-interaction network).

Math: each CIN layer computes, per sample b and feature-dim d (a "column"
n=(b,d)):  y[o] = sum_{h,m} W[o,h,m] * a[h] * b[m]  — a bilinear form.

We avoid materializing the outer-product tensor z[h*m, n] (which needs slow
cross-partition broadcasts) by polarization:  a*b = ((a+b)^2 - a^2 - b^2)/2.
Each layer becomes:  s = V @ t   (pair sums, TensorE)
                     q = s*s     (elementwise square)
                     y = C @ q + G @ t^2   (TensorE, PSUM-accumulated)
with V a 0/1 pair-selection matrix and C,G folded from W host-side (exact).

Layer 0 uses the symmetric fold (741 unordered pairs of 39 features);
layer 1 uses all 64*39=2496 (nh,x) pairs.  Everything on-device is fp16
(inputs/weights) with fp32 PSUM accumulation.

The loop is software-pipelined one tile deep: iteration j runs layer 0 of
tile j and layer 1 of tile j-1, in PE order
    V0(j) | V1(j-1) | C0(j)+G0 | C1(j-1)+G1
so every matmul trails the square it consumes by ~20 matmuls and the
relu->V1 dependency crosses an iteration boundary instead of stalling the
in-order PE queue.

Engine split of the 13 wide square-pairs: 8 on ACT (direct PSUM square),
3 on DVE (copy+multiply), 2 copied by DVE and multiplied by Pool (their C1
matmuls are deferred to the end of the y1 accumulation — Pool is slow).
x^2 / nh^2 / d-sum reduces stay on DVE; relu+bias on ACT.
V outputs are written pairwise into [128,1024] two-bank PSUM tiles so each
square covers two chunks. PSUM: 3x2 (pairs) + 1 (y0) + 1 (y1) = 8 banks.

Sharding: pure data parallel — batch 4096 split as 512 per NeuronCore
across 8 cores; weights replicated.
"""

import numpy as np

B, F, D = 4096, 39, 16
L0, L1 = 128, 128
H1 = L0 // 2                      # 64 hidden maps feed layer 1
NCORES = 8
BL = B // NCORES                  # 512 samples per core
NCOL = BL * D                     # 8192 columns per core
NT = 512                          # columns per tile
NTILES = NCOL // NT               # 16
NB = NT // D                      # samples per tile (32)

K0 = F * (F - 1) // 2             # 741 layer-0 pairs
K1 = H1 * F                       # 2496 layer-1 pairs
NC0 = (K0 + 127) // 128           # 6 chunks (K0 padded to 768)
NC1 = (K1 + 127) // 128           # 20 chunks (K1 padded to 2560)
NP0 = NC0 // 2                    # 3 wide chunk-pairs
NP1 = NC1 // 2                    # 10 wide chunk-pairs
T1 = 128                          # t rows: [x 0:39 | zeros 39:64 | nh 64:128]
NH0 = 64                          # nh base partition in t

PLAN0 = "ADA"                     # square engine per layer-0 pair
PLAN1 = "PADAAAPADA"              # square engine per layer-1 pair


def _host_weights(W0, b0, W1, b1):
    """Fold W0/W1 into the square-trick operands (all exact, fp32)."""
    W0 = np.asarray(W0, np.float32)
    W1 = np.asarray(W1, np.float32)
    S0 = W0.reshape(L0, F, F)
    S0 = (S0 + S0.transpose(0, 2, 1)) / 2
    iu = np.triu_indices(F, 1)                       # 741 (h<m) pairs
    V0 = np.zeros((128 * NC0, F), np.float32)
    V0[np.arange(K0), iu[0]] = 1
    V0[np.arange(K0), iu[1]] = 1
    C0 = np.zeros((L0, 128 * NC0), np.float32)
    C0[:, :K0] = S0[:, iu[0], iu[1]]
    rowsum = S0.sum(2)
    G0 = np.einsum('ohh->oh', S0) * 2 - rowsum       # S[h,h] - sum_{m!=h} S[h,m]

    B1 = W1.reshape(L1, H1, F)
    hh, mm = np.meshgrid(np.arange(H1), np.arange(F), indexing='ij')
    hh, mm = hh.ravel(), mm.ravel()                  # 2496 pairs, h-major
    V1 = np.zeros((128 * NC1, T1), np.float32)
    V1[np.arange(K1), mm] = 1                        # x part at rows 0:39
    V1[np.arange(K1), NH0 + hh] = 1                  # nh part at rows 64:128
    C1 = np.zeros((L1, 128 * NC1), np.float32)
    C1[:, :K1] = B1[:, hh, mm] / 2
    G1 = np.zeros((L1, T1), np.float32)
    G1[:, :F] = -B1.sum(1) / 2                       # coeff on x^2
    G1[:, NH0:] = -B1.sum(2) / 2                     # coeff on nh^2

    def blockT(C):
        # [128, K] -> per-128-column-block transpose: lhsT[k, o] = C[o, base+k]
        L, K = C.shape
        return np.ascontiguousarray(
            C.reshape(L, K // 128, 128).transpose(2, 1, 0).reshape(128, -1)
        )

    return {
        "V0T": V0.T.astype(np.float16),              # [39, 768]
        "V1T": V1.T.astype(np.float16),              # [128, 2560]
        "C0T": blockT(C0).astype(np.float16),        # [128, 768]   (lhsT chunks)
        "C1T": blockT(C1).astype(np.float16),        # [128, 2560]
        "G0T": G0.T.astype(np.float16),              # [39, 128]
        "G1T": G1.T.astype(np.float16),              # [128, 128]
        "b0": np.asarray(b0, np.float32).reshape(L0, 1),
        "b1": np.asarray(b1, np.float32).reshape(L1, 1),
    }


_NC_CACHE = {}


def _build_nc(repeat=1):
    key = ("nc", repeat)
    if key in _NC_CACHE:
        return _NC_CACHE[key]
    from contextlib import ExitStack
    import concourse.bacc as bacc
    import concourse.mybir as mybir
    import concourse.tile as tile

    f16 = mybir.dt.float16
    f32 = mybir.dt.float32

    nc = bacc.Bacc("TRN2", target_bir_lowering=False, debug=False)

    xT_d = nc.dram_tensor("xT", [F, NCOL], f16, kind="ExternalInput")
    V0T_d = nc.dram_tensor("V0T", [F, 128 * NC0], f16, kind="ExternalInput")
    V1T_d = nc.dram_tensor("V1T", [T1, 128 * NC1], f16, kind="ExternalInput")
    C0T_d = nc.dram_tensor("C0T", [128, 128 * NC0], f16, kind="ExternalInput")
    C1T_d = nc.dram_tensor("C1T", [128, 128 * NC1], f16, kind="ExternalInput")
    G0T_d = nc.dram_tensor("G0T", [F, 128], f16, kind="ExternalInput")
    G1T_d = nc.dram_tensor("G1T", [T1, 128], f16, kind="ExternalInput")
    b0_d = nc.dram_tensor("b0", [L0, 1], f32, kind="ExternalInput")
    b1_d = nc.dram_tensor("b1", [L1, 1], f32, kind="ExternalInput")
    out_d = nc.dram_tensor("out", [L0 - H1 + L1, BL], f32, kind="ExternalOutput")

    Relu = mybir.ActivationFunctionType.Relu

    ntot = repeat * NTILES

    with tile.TileContext(nc) as tc, ExitStack() as ctx:
        const = ctx.enter_context(tc.tile_pool(name="const", bufs=1))
        tp = ctx.enter_context(tc.tile_pool(name="tp", bufs=1))
        sqp = ctx.enter_context(tc.tile_pool(name="sqp", bufs=16))
        scrp = ctx.enter_context(tc.tile_pool(name="scrp", bufs=4))
        rp = ctx.enter_context(tc.tile_pool(name="rp", bufs=2))
        outp = ctx.enter_context(tc.tile_pool(name="outp", bufs=1))
        sps = ctx.enter_context(tc.tile_pool(name="sps", bufs=3, space="PSUM"))
        yps0 = ctx.enter_context(tc.tile_pool(name="yps0", bufs=1, space="PSUM"))
        yps1 = ctx.enter_context(tc.tile_pool(name="yps1", bufs=1, space="PSUM"))

        # resident weights: layer-0 set first so tile 0 can start while the
        # (larger) layer-1 weights are still in flight
        V0T = const.tile([F, 128 * NC0], f16)
        C0T = const.tile([128, 128 * NC0], f16)
        G0T = const.tile([F, 128], f16)
        b0t = const.tile([L0, 1], f32)
        V1T = const.tile([T1, 128 * NC1], f16)
        C1T = const.tile([128, 128 * NC1], f16)
        G1T = const.tile([T1, 128], f16)
        b1t = const.tile([L1, 1], f32)

        # persistent t = [x; 0; nh] and t2 = [x^2; 0; nh^2] ring buffers
        t_bufs = [tp.tile([T1, NT], f16, name=f"t{i}", tag=f"t{i}")
                  for i in range(3)]
        t2_bufs = [tp.tile([T1, NT], f16, name=f"t2_{i}", tag=f"t2_{i}")
                   for i in range(3)]

        for dst, src in ((V0T, V0T_d), (C0T, C0T_d), (G0T, G0T_d),
                         (b0t, b0_d)):
            nc.sync.dma_start(out=dst[:], in_=src.ap())
        for tt in (*t_bufs, *t2_bufs):
            nc.vector.memset(tt[32:NH0, :], 0.0)     # one-time zero padding
        # prefetch x for tiles 0 and 1 ahead of the big layer-1 weights
        nc.sync.dma_start(out=t_bufs[0][0:F, :], in_=xT_d.ap()[:, 0:NT])
        nc.sync.dma_start(out=t_bufs[1][0:F, :], in_=xT_d.ap()[:, NT:2 * NT])
        for dst, src in ((V1T, V1T_d), (C1T, C1T_d), (G1T, G1T_d),
                         (b1t, b1_d)):
            nc.sync.dma_start(out=dst[:], in_=src.ap())

        out0 = outp.tile([H1, BL], f32)
        out1 = outp.tile([L1, BL], f32)

        def square(dst, src, how, scr):
            if how == 'A':
                nc.scalar.square(dst, src)
            else:
                nc.vector.tensor_copy(scr[:], src)
                eng = nc.vector if how == 'D' else nc.gpsimd
                eng.tensor_mul(dst, scr[:], scr[:])

        def v_pairs(npairs, VT, trhs, plan):
            sqs = []
            for p in range(npairs):
                ps = sps.tile([128, 2 * NT], f32, name="ps")
                for h in range(2):
                    i = 2 * p + h
                    nc.tensor.matmul(ps[:, h * NT:(h + 1) * NT],
                                     VT[:, i * 128:(i + 1) * 128],
                                     trhs, start=True, stop=True)
                sq = sqp.tile([128, 2 * NT], f16, name="sq")
                scr = (scrp.tile([128, 2 * NT], f16, name="scr")
                       if plan[p] != 'A' else None)
                square(sq[:], ps[:], plan[p], scr)
                sqs.append(sq)
            return sqs

        # per-tile state carried across the pipelined loop
        sq1_of = {}

        for j in range(ntot + 1):
            nt = j % NTILES
            pv = (j - 1) % NTILES                    # previous tile index
            t = t_bufs[j % 3]
            t2 = t2_bufs[j % 3]
            tprev = t_bufs[(j - 1) % 3]
            t2prev = t2_bufs[(j - 1) % 3]

            if j < ntot:
                if j + 1 < ntot:                     # prefetch next x tile
                    nxt = (j + 1) % NTILES
                    nc.sync.dma_start(
                        out=t_bufs[(j + 1) % 3][0:F, :],
                        in_=xT_d.ap()[:, nxt * NT:(nxt + 1) * NT])
                nc.vector.tensor_mul(t2[0:F, :], t[0:F, :], t[0:F, :])  # x^2
                # layer 0 pair sums + squares for tile j
                sq0 = v_pairs(NP0, V0T, t[0:F, :], PLAN0)

            if j >= 1:
                # layer 1 pair sums + squares for tile j-1
                sq1_of[j - 1] = v_pairs(NP1, V1T, tprev[:], PLAN1)

            if j < ntot:
                # y0(j) = G0 @ x^2 + C0 @ q0   (G0 first: x^2 ready early)
                y0 = yps0.tile([L0, NT], f32)
                nc.tensor.matmul(y0[:], G0T[:], t2[0:F, :],
                                 start=True, stop=False)
                for i in range(NC0):
                    nc.tensor.matmul(
                        y0[:], C0T[:, i * 128:(i + 1) * 128],
                        sq0[i // 2][:, (i % 2) * NT:(i % 2 + 1) * NT],
                        start=False, stop=(i == NC0 - 1))
                # relu + split: nh feeds layer 1 next iteration
                nc.scalar.activation(t[NH0:T1, :], y0[0:H1, :], Relu,
                                     bias=b0t[0:H1])
                r0 = rp.tile([H1, NT], f32, tag="r0")
                nc.scalar.activation(r0[:], y0[H1:L0, :], Relu,
                                     bias=b0t[H1:L0])
                nc.vector.tensor_mul(t2[NH0:T1, :], t[NH0:T1, :],
                                     t[NH0:T1, :])  # nh^2
                nc.vector.tensor_reduce(
                    out0[:, nt * NB:(nt + 1) * NB],
                    r0[:].rearrange("p (b d) -> p b d", d=D),
                    axis=mybir.AxisListType.X, op=mybir.AluOpType.add)

            if j >= 1:
                # y1(j-1) = C1 @ q1 + G1 @ t^2   (Pool-squared chunks last)
                sq1 = sq1_of.pop(j - 1)
                y1 = yps1.tile([L1, NT], f32)
                order = [i for i in range(NC1) if PLAN1[i // 2] != 'P']
                order += [i for i in range(NC1) if PLAN1[i // 2] == 'P']
                for n, i in enumerate(order):
                    nc.tensor.matmul(
                        y1[:], C1T[:, i * 128:(i + 1) * 128],
                        sq1[i // 2][:, (i % 2) * NT:(i % 2 + 1) * NT],
                        start=(n == 0), stop=(n == NC1 - 1))
                    if n == 0:
                        nc.tensor.matmul(y1[:], G1T[:], t2prev[:],
                                         start=False, stop=False)
                r1 = rp.tile([L1, NT], f32, tag="r1")
                nc.scalar.activation(r1[:], y1[:], Relu, bias=b1t[:])
                nc.vector.tensor_reduce(
                    out1[:, pv * NB:(pv + 1) * NB],
                    r1[:].rearrange("p (b d) -> p b d", d=D),
                    axis=mybir.AxisListType.X, op=mybir.AluOpType.add)

        nc.sync.dma_start(out=out_d.ap()[0:H1, :], in_=out0[:])
        nc.sync.dma_start(out=out_d.ap()[H1:, :], in_=out1[:])

    nc.compile()
    _NC_CACHE[key] = nc
    return nc


def _run(inputs, trace=False):
    from concourse.bass_utils import run_bass_kernel_spmd

    x = np.asarray(inputs["x"], np.float32)
    w = _host_weights(inputs["W0"], inputs["b0"], inputs["W1"], inputs["b1"])
    nc = _build_nc()

    in_maps = []
    for c in range(NCORES):
        xs = x[c * BL:(c + 1) * BL]                          # [512, 39, 16]
        xT = np.ascontiguousarray(
            xs.transpose(1, 0, 2).reshape(F, NCOL)).astype(np.float16)
        m = {"xT": xT}
        m.update(w)
        in_maps.append(m)

    res = run_bass_kernel_spmd(nc, in_maps, core_ids=list(range(NCORES)),
                               trace=trace)
    out = np.empty((B, L0 - H1 + L1), np.float32)
    for c in range(NCORES):
        out[c * BL:(c + 1) * BL] = res.results[c]["out"].T
    return out, res


def kernel(**inputs):
    out, _ = _run(inputs)
    return out


# revision 23
# speedup vs baseline: 1.9457x; 1.0116x over previous
"""Trainium2 Bass kernel for nn_CIN (xDeepFM compressed-interaction network).

Math: each CIN layer computes, per sample b and feature-dim d (a "column"
n=(b,d)):  y[o] = sum_{h,m} W[o,h,m] * a[h] * b[m]  — a bilinear form.

We avoid materializing the outer-product tensor z[h*m, n] (which needs slow
cross-partition broadcasts) by polarization:  a*b = ((a+b)^2 - a^2 - b^2)/2.
Each layer becomes:  s = V @ t   (pair sums, TensorE)
                     q = s*s     (elementwise square)
                     y = C @ q + G @ t^2   (TensorE, PSUM-accumulated)
with V a 0/1 pair-selection matrix and C,G folded from W host-side (exact).

Layer 0 uses the symmetric fold (741 unordered pairs of 39 features);
layer 1 uses all 64*39=2496 (nh,x) pairs.  Everything on-device is fp16
(inputs/weights) with fp32 PSUM accumulation.

The loop is software-pipelined one tile deep: iteration j runs layer 0 of
tile j and layer 1 of tile j-1, in PE order
    V0(j) | V1(j-1) | C0(j)+G0 | C1(j-1)+G1
so every matmul trails the square it consumes by ~20 matmuls and the
relu->V1 dependency crosses an iteration boundary instead of stalling the
in-order PE queue.

Engine split of the 13 wide square-pairs: 8 on ACT (direct PSUM square),
3 on DVE (copy+multiply), 2 copied by DVE and multiplied by Pool (their C1
matmuls are deferred to the end of the y1 accumulation — Pool is slow).
x^2 / nh^2 / d-sum reduces stay on DVE; relu+bias on ACT.
V outputs are written pairwise into [128,1024] two-bank PSUM tiles so each
square covers two chunks. PSUM: 3x2 (pairs) + 1 (y0) + 1 (y1) = 8 banks.

Sharding: pure data parallel — batch 4096 split as 512 per NeuronCore
across 8 cores; weights replicated.
"""

import numpy as np

B, F, D = 4096, 39, 16
L0, L1 = 128, 128
H1 = L0 // 2                      # 64 hidden maps feed layer 1
NCORES = 8
BL = B // NCORES                  # 512 samples per core
NCOL = BL * D                     # 8192 columns per core
NT = 512                          # columns per tile
NTILES = NCOL // NT               # 16
NB = NT // D                      # samples per tile (32)

K0 = F * (F - 1) // 2             # 741 layer-0 pairs
K1 = H1 * F                       # 2496 layer-1 pairs
NC0 = (K0 + 127) // 128           # 6 chunks (K0 padded to 768)
NC1 = (K1 + 127) // 128           # 20 chunks (K1 padded to 2560)
NP0 = NC0 // 2                    # 3 wide chunk-pairs
NP1 = NC1 // 2                    # 10 wide chunk-pairs
T1 = 128                          # t rows: [x 0:39 | zeros 39:64 | nh 64:128]
NH0 = 64                          # nh base partition in t

PLAN0 = "ADA"                     # square engine per layer-0 pair
PLAN1 = "PADAAAPADA"              # square engine per layer-1 pair


def _host_weights(W0, b0, W1, b1):
    """Fold W0/W1 into the square-trick operands (all exact, fp32)."""
    W0 = np.asarray(W0, np.float32)
    W1 = np.asarray(W1, np.float32)
    S0 = W0.reshape(L0, F, F)
    S0 = (S0 + S0.transpose(0, 2, 1)) / 2
    iu = np.triu_indices(F, 1)                       # 741 (h<m) pairs
    V0 = np.zeros((128 * NC0, F), np.float32)
    V0[np.arange(K0), iu[0]] = 1
    V0[np.arange(K0), iu[1]] = 1
    C0 = np.zeros((L0, 128 * NC0), np.float32)
    C0[:, :K0] = S0[:, iu[0], iu[1]]
    rowsum = S0.sum(2)
    G0 = np.einsum('ohh->oh', S0) * 2 - rowsum       # S[h,h] - sum_{m!=h} S[h,m]

    B1 = W1.reshape(L1, H1, F)
    hh, mm = np.meshgrid(np.arange(H1), np.arange(F), indexing='ij')
    hh, mm = hh.ravel(), mm.ravel()                  # 2496 pairs, h-major
    V1 = np.zeros((128 * NC1, T1), np.float32)
    V1[np.arange(K1), mm] = 1                        # x part at rows 0:39
    V1[np.arange(K1), NH0 + hh] = 1                  # nh part at rows 64:128
    C1 = np.zeros((L1, 128 * NC1), np.float32)
    C1[:, :K1] = B1[:, hh, mm] / 2
    G1 = np.zeros((L1, T1), np.float32)
    G1[:, :F] = -B1.sum(1) / 2                       # coeff on x^2
    G1[:, NH0:] = -B1.sum(2) / 2                     # coeff on nh^2

    def blockT(C):
        # [128, K] -> per-128-column-block transpose: lhsT[k, o] = C[o, base+k]
        L, K = C.shape
        return np.ascontiguousarray(
            C.reshape(L, K // 128, 128).transpose(2, 1, 0).reshape(128, -1)
        )

    w0p = np.zeros((128, 128 * NC0 * 2 + 128), np.float16)
    w0p[:F, :128 * NC0] = V0.T
    w0p[:, 128 * NC0:128 * NC0 * 2] = blockT(C0)
    w0p[:F, 128 * NC0 * 2:] = G0.T
    w1p = np.zeros((128, 128 * NC1 * 2 + 128), np.float16)
    w1p[:, :128 * NC1] = V1.T
    w1p[:, 128 * NC1:128 * NC1 * 2] = blockT(C1)
    w1p[:, 128 * NC1 * 2:] = G1.T
    bp = np.stack([np.asarray(b0, np.float32),
                   np.asarray(b1, np.float32)], 1)
    return {"w0p": w0p, "w1p": w1p, "bp": bp.astype(np.float32)}


_NC_CACHE = {}


def _build_nc(repeat=1):
    key = ("nc", repeat)
    if key in _NC_CACHE:
        return _NC_CACHE[key]
    from contextlib import ExitStack
    import concourse.bacc as bacc
    import concourse.mybir as mybir
    import concourse.tile as tile

    f16 = mybir.dt.float16
    f32 = mybir.dt.float32

    nc = bacc.Bacc("TRN2", target_bir_lowering=False, debug=False)

    xT_d = nc.dram_tensor("xT", [F, NCOL], f16, kind="ExternalInput")
    w0p_d = nc.dram_tensor("w0p", [128, 128 * NC0 * 2 + 128], f16,
                           kind="ExternalInput")
    w1p_d = nc.dram_tensor("w1p", [128, 128 * NC1 * 2 + 128], f16,
                           kind="ExternalInput")
    bp_d = nc.dram_tensor("bp", [128, 2], f32, kind="ExternalInput")
    out_d = nc.dram_tensor("out", [L0 - H1 + L1, BL], f32, kind="ExternalOutput")

    Relu = mybir.ActivationFunctionType.Relu

    ntot = repeat * NTILES

    with tile.TileContext(nc) as tc, ExitStack() as ctx:
        const = ctx.enter_context(tc.tile_pool(name="const", bufs=1))
        tp = ctx.enter_context(tc.tile_pool(name="tp", bufs=1))
        sqp = ctx.enter_context(tc.tile_pool(name="sqp", bufs=16))
        scrp = ctx.enter_context(tc.tile_pool(name="scrp", bufs=4))
        rp = ctx.enter_context(tc.tile_pool(name="rp", bufs=2))
        outp = ctx.enter_context(tc.tile_pool(name="outp", bufs=1))
        sps = ctx.enter_context(tc.tile_pool(name="sps", bufs=3, space="PSUM"))
        yps0 = ctx.enter_context(tc.tile_pool(name="yps0", bufs=1, space="PSUM"))
        yps1 = ctx.enter_context(tc.tile_pool(name="yps1", bufs=1, space="PSUM"))

        # resident weights, packed: layer-0 set first so tile 0 can start
        # while the (larger) layer-1 weights are still in flight
        W0P = const.tile([128, 128 * NC0 * 2 + 128], f16)
        W1P = const.tile([128, 128 * NC1 * 2 + 128], f16)
        BP = const.tile([128, 2], f32)
        V0T = W0P[0:F, 0:128 * NC0]
        C0T = W0P[:, 128 * NC0:128 * NC0 * 2]
        G0T = W0P[0:F, 128 * NC0 * 2:]
        V1T = W1P[:, 0:128 * NC1]
        C1T = W1P[:, 128 * NC1:128 * NC1 * 2]
        G1T = W1P[:, 128 * NC1 * 2:]
        b0t = BP[:, 0:1]
        b1t = BP[:, 1:2]

        # persistent t = [x; 0; nh] and t2 = [x^2; 0; nh^2] ring buffers
        t_bufs = [tp.tile([T1, NT], f16, name=f"t{i}", tag=f"t{i}")
                  for i in range(3)]
        t2_bufs = [tp.tile([T1, NT], f16, name=f"t2_{i}", tag=f"t2_{i}")
                   for i in range(3)]

        nc.sync.dma_start(out=W0P[:], in_=w0p_d.ap())
        nc.sync.dma_start(out=BP[:], in_=bp_d.ap())
        for tt in (*t_bufs, *t2_bufs):
            nc.vector.memset(tt[32:NH0, :], 0.0)     # one-time zero padding
        # prefetch x for tiles 0 and 1 ahead of the big layer-1 weights
        nc.sync.dma_start(out=t_bufs[0][0:F, :], in_=xT_d.ap()[:, 0:NT])
        nc.sync.dma_start(out=t_bufs[1][0:F, :], in_=xT_d.ap()[:, NT:2 * NT])
        nc.sync.dma_start(out=W1P[:], in_=w1p_d.ap())

        out0 = outp.tile([H1, BL], f32)
        out1 = outp.tile([L1, BL], f32)

        def square(dst, src, how, scr):
            if how == 'A':
                nc.scalar.square(dst, src)
            else:
                nc.vector.tensor_copy(scr[:], src)
                eng = nc.vector if how == 'D' else nc.gpsimd
                eng.tensor_mul(dst, scr[:], scr[:])

        def v_pairs(npairs, VT, trhs, plan):
            sqs = []
            for p in range(npairs):
                ps = sps.tile([128, 2 * NT], f32, name="ps")
                for h in range(2):
                    i = 2 * p + h
                    nc.tensor.matmul(ps[:, h * NT:(h + 1) * NT],
                                     VT[:, i * 128:(i + 1) * 128],
                                     trhs, start=True, stop=True)
                sq = sqp.tile([128, 2 * NT], f16, name="sq")
                scr = (scrp.tile([128, 2 * NT], f16, name="scr")
                       if plan[p] != 'A' else None)
                square(sq[:], ps[:], plan[p], scr)
                sqs.append(sq)
            return sqs

        # per-tile state carried across the pipelined loop
        sq1_of = {}

        for j in range(ntot + 1):
            nt = j % NTILES
            pv = (j - 1) % NTILES                    # previous tile index
            t = t_bufs[j % 3]
            t2 = t2_bufs[j % 3]
            tprev = t_bufs[(j - 1) % 3]
            t2prev = t2_bufs[(j - 1) % 3]

            if j < ntot:
                if j + 1 < ntot:                     # prefetch next x tile
                    nxt = (j + 1) % NTILES
                    nc.sync.dma_start(
                        out=t_bufs[(j + 1) % 3][0:F, :],
                        in_=xT_d.ap()[:, nxt * NT:(nxt + 1) * NT])
                nc.vector.tensor_mul(t2[0:F, :], t[0:F, :], t[0:F, :])  # x^2
                # layer 0 pair sums + squares for tile j
                sq0 = v_pairs(NP0, V0T, t[0:F, :], PLAN0)

            if j >= 1:
                # layer 1 pair sums + squares for tile j-1
                sq1_of[j - 1] = v_pairs(NP1, V1T, tprev[:], PLAN1)

            if j < ntot:
                # y0(j) = G0 @ x^2 + C0 @ q0   (G0 first: x^2 ready early)
                y0 = yps0.tile([L0, NT], f32)
                nc.tensor.matmul(y0[:], G0T[:], t2[0:F, :],
                                 start=True, stop=False)
                for i in range(NC0):
                    nc.tensor.matmul(
                        y0[:], C0T[:, i * 128:(i + 1) * 128],
                        sq0[i // 2][:, (i % 2) * NT:(i % 2 + 1) * NT],
                        start=False, stop=(i == NC0 - 1))
                # relu + split: nh feeds layer 1 next iteration
                nc.scalar.activation(t[NH0:T1, :], y0[0:H1, :], Relu,
                                     bias=b0t[0:H1])
                r0 = rp.tile([H1, NT], f32, tag="r0")
                nc.scalar.activation(r0[:], y0[H1:L0, :], Relu,
                                     bias=b0t[H1:L0])
                nc.vector.tensor_mul(t2[NH0:T1, :], t[NH0:T1, :],
                                     t[NH0:T1, :])  # nh^2
                nc.vector.tensor_reduce(
                    out0[:, nt * NB:(nt + 1) * NB],
                    r0[:].rearrange("p (b d) -> p b d", d=D),
                    axis=mybir.AxisListType.X, op=mybir.AluOpType.add)

            if j >= 1:
                # y1(j-1) = C1 @ q1 + G1 @ t^2   (Pool-squared chunks last)
                sq1 = sq1_of.pop(j - 1)
                y1 = yps1.tile([L1, NT], f32)
                order = [i for i in range(NC1) if PLAN1[i // 2] != 'P']
                order += [i for i in range(NC1) if PLAN1[i // 2] == 'P']
                for n, i in enumerate(order):
                    nc.tensor.matmul(
                        y1[:], C1T[:, i * 128:(i + 1) * 128],
                        sq1[i // 2][:, (i % 2) * NT:(i % 2 + 1) * NT],
                        start=(n == 0), stop=(n == NC1 - 1))
                    if n == 0:
                        nc.tensor.matmul(y1[:], G1T[:], t2prev[:],
                                         start=False, stop=False)
                r1 = rp.tile([L1, NT], f32, tag="r1")
                nc.scalar.activation(r1[:], y1[:], Relu, bias=b1t[:])
                nc.vector.tensor_reduce(
                    out1[:, pv * NB:(pv + 1) * NB],
                    r1[:].rearrange("p (b d) -> p b d", d=D),
                    axis=mybir.AxisListType.X, op=mybir.AluOpType.add)
                if ntot == NTILES and j == NTILES // 2:
                    # flush the finished first half of the output early so the
                    # final drain only waits on the second half
                    hb = (NTILES // 2) * NB
                    nc.sync.dma_start(out=out_d.ap()[0:H1, 0:hb],
                                      in_=out0[:, 0:hb])
                    nc.sync.dma_start(out=out_d.ap()[H1:, 0:hb],
                                      in_=out1[:, 0:hb])

        if ntot == NTILES:
            hb = (NTILES // 2) * NB
            nc.sync.dma_start(out=out_d.ap()[0:H1, hb:], in_=out0[:, hb:])
            nc.sync.dma_start(out=out_d.ap()[H1:, hb:], in_=out1[:, hb:])
        else:
            nc.sync.dma_start(out=out_d.ap()[0:H1, :], in_=out0[:])
            nc.sync.dma_start(out=out_d.ap()[H1:, :], in_=out1[:])

    nc.compile()
    _NC_CACHE[key] = nc
    return nc


def _run(inputs, trace=False):
    from concourse.bass_utils import run_bass_kernel_spmd

    x = np.asarray(inputs["x"], np.float32)
    w = _host_weights(inputs["W0"], inputs["b0"], inputs["W1"], inputs["b1"])
    nc = _build_nc()

    in_maps = []
    for c in range(NCORES):
        xs = x[c * BL:(c + 1) * BL]                          # [512, 39, 16]
        xT = np.ascontiguousarray(
            xs.transpose(1, 0, 2).reshape(F, NCOL)).astype(np.float16)
        m = {"xT": xT}
        m.update(w)
        in_maps.append(m)

    res = run_bass_kernel_spmd(nc, in_maps, core_ids=list(range(NCORES)),
                               trace=trace)
    out = np.empty((B, L0 - H1 + L1), np.float32)
    for c in range(NCORES):
        out[c * BL:(c + 1) * BL] = res.results[c]["out"].T
    return out, res


def kernel(**inputs):
    out, _ = _run(inputs)
    return out
